# revision 1
# baseline (speedup 1.0000x reference)
"""Causal multi-head attention (B=4, T=2048, H=1024, 16 heads) on 8 trn2 cores.

Sharding: batch(4) x head-group(2).  Core c -> batch b=c//2, heads g=c%2
(8 heads each): zero-communication data/tensor parallelism.  Each core
computes its QKV projection slice, causal+padding-masked attention for its 8
heads, and a row-parallel slice of the output projection; the two partial
outputs per batch row are summed on the host (row-parallel unshard).

Device algorithm (per core; attention kept transposed so softmax reduces
along the PE contraction dim, all matmuls at 1 cycle/row):
  xT [H, T] bf16 (host-pretransposed input row)
  QT/KT [512, T] bf16 = wqk^T-slices @ xT  (Q pre-scaled by 1/sqrt(hd) on
      host; bias added on DVE during the PSUM->SBUF move, per-partition AP)
  V [T, 8x65] bf16 = xT^T @ wv, bias via DVE add of a gpsimd-broadcast
      replicated row; a ones column per head; all 65 columns multiplied by
      the key-padding 0/1 mask (per-partition scalar) -> padded keys drop
      out of both the attention numerator and the softmax denominator, so
      the exp needs no mask bias at all.
  per (head, q-tile 512, k-chunk pair 2x128):
      S^T[k, q] = KT_h[:, chunk].T @ QT_h[:, qtile]     (bf16, f32 PSUM)
      P^T = exp(S^T)         (ScalarE, [128,1024] two-chunk ops, bias 0.0)
      causal masking on diagonal chunks: P^T *= 0/1 mask on DVE (post-exp,
      off the ScalarE critical path; fully-masked chunks never computed)
      o^T[65, q] += V_aug[chunk, head].T @ P^T          (row 64 = denom)
  whole-head S streams with the exps trailing on ScalarE, then the dense PV
  stream; V-projection chunks 4..15 and the previous q-tile's output
  projection are braided into the attention stream as PE filler so the HAM
  clock gate stays at 2.4GHz.
  o_scaled = o^T[0:64] * (1/denom) (DVE approx recip, gpsimd broadcast),
  written/DMA-shifted into dense head-pair tiles [128, 512]
  y[t, j] = sum_hp o_dense_hp[:, t].T @ wout_hp[:, j] (f32r) + b_out on DVE
"""

import os
import sys

import numpy as np

sys.path.insert(0, "/opt/trn_rl_repo")

B, T, H = 4, 2048, 1024
NH, HD = 16, 64
NCORES = 8
HPC = 8          # heads per core
GD = HPC * HD    # head dims per core = 512
KC = T // 128    # 16 k-chunks
QT_TILES = T // 512  # 4 q-tiles
HC = H // 128    # 8 h-chunks (contraction for projections)

NEG = -1.0e9


def _build_nc():
    import concourse.bass as bass
    import concourse.tile as tile
    import concourse.mybir as mybir
    from concourse import bacc
    from contextlib import ExitStack

    f32 = mybir.dt.float32
    f32r = mybir.dt.float32r
    bf16 = mybir.dt.bfloat16
    EXP = mybir.ActivationFunctionType.Exp

    nc = bacc.Bacc("TRN2", target_bir_lowering=False, debug=False)

    xT_d = nc.dram_tensor("xT", [H, T], bf16, kind="ExternalInput").ap()
    wqk_d = nc.dram_tensor("wqk", [H, 2 * GD], bf16, kind="ExternalInput").ap()
    wv_d = nc.dram_tensor("wv", [H, GD], bf16, kind="ExternalInput").ap()
    bqkc_d = nc.dram_tensor("bqkc", [128, 8], f32, kind="ExternalInput").ap()
    bv_d = nc.dram_tensor("bv", [1, GD], f32, kind="ExternalInput").ap()
    wout_d = nc.dram_tensor("wout", [GD, H], f32, kind="ExternalInput").ap()
    bout_d = nc.dram_tensor("bout", [1, H], f32, kind="ExternalInput").ap()
    padb01_d = nc.dram_tensor("padb01", [128, KC], f32, kind="ExternalInput").ap()
    cmask_d = nc.dram_tensor("cmask", [128, 4 * 512], bf16, kind="ExternalInput").ap()
    y_d = nc.dram_tensor("y", [T, H], f32, kind="ExternalOutput").ap()

    def r(ap):
        return ap.bitcast(f32r)

    def emit_v_proj(nc, r, bvrep, xt, wvts, psv, v_sb, padb01_sb,
                    ts, HC, HPC):
        for hc in range(HC):
            nc.tensor.matmul(
                psv, xt[hc][:, ts * 128:(ts + 1) * 128], wvts[hc],
                start=(hc == 0), stop=(hc == HC - 1))
        pad_c = padb01_sb[:, ts:ts + 1]
        dst = v_sb[ts].rearrange("p (h c) -> p h c", h=HPC)[:, :, 0:64]
        bsrc = bvrep.rearrange("p (h c) -> p h c", h=HPC)
        srcv = psv.rearrange("p (h c) -> p h c", h=HPC)
        nc.vector.tensor_add(dst, bsrc, srcv)
        nc.vector.tensor_scalar_mul(dst, dst, pad_c)
        onescols = v_sb[ts].rearrange("p (h c) -> p h c", h=HPC)[:, :, 64:65]
        nc.vector.memset(onescols, 1.0)
        nc.vector.tensor_scalar_mul(onescols, onescols, pad_c)

    with ExitStack() as ctx:
        tc = ctx.enter_context(tile.TileContext(nc))

        const = ctx.enter_context(tc.tile_pool(name="const", bufs=1))
        padb01_sb = const.tile([128, KC], f32, name="padb01_sb")
        nc.sync.dma_start(padb01_sb, padb01_d)

        # Persistent activations
        acts = ctx.enter_context(tc.tile_pool(name="acts", bufs=1))
        qk_sb = [acts.tile([128, T], bf16, name=f"qk{i}") for i in range(8)]
        v_sb = [acts.tile([128, HPC * 65], bf16, name=f"v{c}") for c in range(KC)]

        # xt / wv / small consts stay resident through phase 2 (the V
        # projection of chunks 4..15 is braided into the attention stream).
        p1c = ctx.enter_context(tc.tile_pool(name="p1c", bufs=1))
        bqkc_sb = p1c.tile([128, 8], f32, name="bqkc_sb")
        nc.sync.dma_start(bqkc_sb, bqkc_d)
        bv_sb = p1c.tile([1, GD], f32, name="bv_sb")
        nc.sync.dma_start(bv_sb, bv_d)
        bvrep = p1c.tile([128, GD], f32, name="bvrep")
        nc.gpsimd.partition_broadcast(bvrep, bv_sb)

        xt_pool = ctx.enter_context(tc.tile_pool(name="xt", bufs=1))
        xt = [xt_pool.tile([128, T], bf16, name=f"xt{i}") for i in range(HC)]
        for i in range(HC):
            nc.sync.dma_start(xt[i], xT_d[i * 128:(i + 1) * 128, :])
        wv_pool = ctx.enter_context(tc.tile_pool(name="wvp", bufs=8))
        wvts = []
        for hc in range(HC):
            wvt = wv_pool.tile([128, GD], bf16, tag="wv", name=f"wv{hc}")
            nc.sync.dma_start(wvt, wv_d[hc * 128:(hc + 1) * 128, :])
            wvts.append(wvt)

        # ---------------- Phase 1: QK projections + V chunks 0..3 ----------
        with ExitStack() as p1:
            wqk_pool = p1.enter_context(tc.tile_pool(name="wqkp", bufs=16))
            ps1 = p1.enter_context(tc.tile_pool(name="ps1", bufs=4, space="PSUM"))

            # Q^T and K^T: out[col, t] tiles
            for ct in range(8):
                wts = []
                for hc in range(HC):
                    wt = wqk_pool.tile([128, 128], bf16, tag="w", name=f"w{ct}_{hc}")
                    nc.sync.dma_start(
                        wt, wqk_d[hc * 128:(hc + 1) * 128, ct * 128:(ct + 1) * 128])
                    wts.append(wt)
                for tt in range(4):
                    ps = ps1.tile([128, 512], f32, tag="ps", name=f"psqk{ct}_{tt}")
                    for hc in range(HC):
                        nc.tensor.matmul(
                            ps, wts[hc], xt[hc][:, tt * 512:(tt + 1) * 512],
                            start=(hc == 0), stop=(hc == HC - 1))
                    nc.vector.tensor_scalar_add(
                        qk_sb[ct][:, tt * 512:(tt + 1) * 512], ps,
                        bqkc_sb[:, ct:ct + 1])

            # V chunks 0..3 (needed by qt0's PV); the rest are braided into
            # the attention stream as PE filler.
            for ts in range(4):
                psv = ps1.tile([128, 512], f32, tag="psv", name=f"psv{ts}")
                emit_v_proj(nc, r, bvrep, xt, wvts, psv, v_sb,
                            padb01_sb, ts, HC, HPC)

        # ---------------- Phase 2: attention + output projection ----------------
        with ExitStack() as p2:
            p2c = p2.enter_context(tc.tile_pool(name="p2c", bufs=1))
            cmask_sb = p2c.tile([128, 4 * 512], bf16, name="cmask_sb")
            nc.sync.dma_start(cmask_sb, cmask_d)
            bout_sb = p2c.tile([1, H], f32, name="bout_sb")
            nc.sync.dma_start(bout_sb, bout_d)
            brep = p2c.tile([128, H], f32, name="brep")
            nc.gpsimd.partition_broadcast(brep, bout_sb)
            wout_sb = [p2c.tile([128, H], f32, name=f"wo{hp}") for hp in range(4)]
            for hp in range(4):
                nc.sync.dma_start(r(wout_sb[hp]), r(wout_d[hp * 128:(hp + 1) * 128, :]))

            ppool = p2.enter_context(tc.tile_pool(name="pchunks", bufs=14))
            osc_pool = p2.enter_context(tc.tile_pool(name="osc", bufs=2))
            oden_pool = p2.enter_context(tc.tile_pool(name="oden", bufs=8))
            dpool = p2.enter_context(tc.tile_pool(name="dtiles", bufs=3))
            ypool = p2.enter_context(tc.tile_pool(name="ysb", bufs=3))
            ps_s = p2.enter_context(tc.tile_pool(name="ps_s", bufs=2, space="PSUM"))
            ps_o = p2.enter_context(tc.tile_pool(name="ps_o", bufs=2, space="PSUM"))
            ps_y = p2.enter_context(tc.tile_pool(name="ps_y", bufs=2, space="PSUM"))

            def attn_tail(qt, h, opsum, o_dense):
                """softmax denom -> recip -> broadcast -> scale -> dense repack"""
                stage = dpool.tile([65, 512], f32, tag="dstage", name=f"st{qt}_{h}")
                nc.vector.tensor_copy(stage[64:65, :], opsum[64:65, :])
                dp0 = dpool.tile([1, 512], f32, tag="dp0", name=f"dp0_{qt}_{h}")
                nc.sync.dma_start(dp0, stage[64:65, :])
                rp0 = dpool.tile([1, 512], f32, tag="rp0", name=f"rp0_{qt}_{h}")
                nc.vector.reciprocal_approx_fast(rp0, dp0)
                rrep = dpool.tile([64, 512], f32, tag="rrep", name=f"rr{qt}_{h}")
                nc.gpsimd.partition_broadcast(rrep, rp0)
                if h % 2 == 0:
                    nc.vector.tensor_mul(r(o_dense[0:64, :]), rrep, opsum[0:64, :])
                else:
                    o_sc = osc_pool.tile([64, 512], f32, tag="osc", name=f"osc{qt}_{h}")
                    nc.vector.tensor_mul(o_sc, rrep, opsum[0:64, :])
                    nc.sync.dma_start(r(o_dense[64:128, :]), r(o_sc))

            def emit_y_tile(qt, j, ts, oden):
                """one output-projection tile for q-tile qt (b_out via preload)"""
                q0 = qt * 512
                ypsum = ps_y.tile([128, 512], f32, tag="y", name=f"y{qt}_{j}_{ts}")
                for hp in range(4):
                    nc.tensor.matmul(
                        ypsum,
                        r(oden[hp][:, ts * 128:(ts + 1) * 128]),
                        r(wout_sb[hp][:, j * 512:(j + 1) * 512]),
                        start=(hp == 0), stop=(hp == 3))
                ysb = ypool.tile([128, 512], f32, tag="ysb", name=f"ys{qt}_{j}_{ts}")
                nc.vector.tensor_add(ysb, brep[:, j * 512:(j + 1) * 512], ypsum)
                nc.sync.dma_start(
                    y_d[q0 + ts * 128:q0 + (ts + 1) * 128, j * 512:(j + 1) * 512],
                    ysb)

            pending_y = []   # deferred output-projection tiles of the prev q-tile
            deferred_v = list(range(4, KC))   # V chunks braided as PE filler

            for qt in range(QT_TILES):
                q0 = qt * 512
                nk = 4 * (qt + 1)
                oden = []
                for h in range(HPC):
                    if h % 2 == 0:
                        o_dense = oden_pool.tile([128, 512], f32, tag="od",
                                                 name=f"od{qt}_{h // 2}")
                        oden.append(o_dense)
                    hq = qk_sb[h // 2][(h % 2) * 64:(h % 2) * 64 + 64, q0:q0 + 512]
                    # S^T in two-chunk psum tiles, whole-head S stream first
                    # (exp trails on ScalarE with small frequent PE waits that
                    # don't trip the HAM throttle), then the dense PV stream.
                    pts = []
                    for cc in range(nk // 2):
                        spsum = ps_s.tile([128, 1024], f32, tag="s",
                                          name=f"s{qt}_{h}_{cc}")
                        for ci in range(2):
                            c = 2 * cc + ci
                            out = spsum[:, ci * 512:(ci + 1) * 512]
                            hk = qk_sb[4 + h // 2][(h % 2) * 64:(h % 2) * 64 + 64,
                                                   c * 128:(c + 1) * 128]
                            nc.tensor.matmul(out, hk, hq, start=True, stop=True)
                        pt = ppool.tile([128, 1024], bf16, tag="p",
                                        name=f"p{qt}_{h}_{cc}")
                        nc.scalar.activation(pt, spsum, EXP, bias=0.0, scale=1.0)
                        for ci in range(2):
                            c = 2 * cc + ci
                            if c >= 4 * qt:
                                dd = c - 4 * qt
                                sl = pt[:, ci * 512:(ci + 1) * 512]
                                nc.vector.tensor_mul(
                                    sl, cmask_sb[:, dd * 512:(dd + 1) * 512], sl)
                        pts.append(pt)
                        if deferred_v:
                            ts_v = deferred_v.pop(0)
                            psv = ps_y.tile([128, 512], f32, tag="y",
                                            name=f"psvd{ts_v}")
                            emit_v_proj(nc, r, bvrep, xt, wvts, psv,
                                        v_sb, padb01_sb, ts_v, HC, HPC)
                        elif cc == 1 and pending_y:
                            pending_y.pop(0)()
                    opsum = ps_o.tile([65, 512], f32, tag="o", name=f"o{qt}_{h}")
                    for c in range(nk):
                        nc.tensor.matmul(
                            opsum,
                            v_sb[c][:, h * 65:(h + 1) * 65].bitcast(bf16),
                            pts[c // 2][:, (c % 2) * 512:(c % 2) * 512 + 512],
                            start=(c == 0), stop=(c == nk - 1))
                    attn_tail(qt, h, opsum, o_dense)

                for j in range(2):
                    for ts in range(4):
                        pending_y.append(
                            lambda qt=qt, j=j, ts=ts, oden=oden: emit_y_tile(qt, j, ts, oden))

            for fn in pending_y:
                fn()

    nc.compile()
    return nc


_NC_CACHE = None


def _get_nc():
    global _NC_CACHE
    if _NC_CACHE is None:
        _NC_CACHE = _build_nc()
    return _NC_CACHE


def make_core_inputs(input, mask, w_qkv, b_qkv, w_out, b_out, core):
    """Host-side sharding/layout prep for one core."""
    b, g = core // 2, core % 2
    scale = 1.0 / np.sqrt(HD)

    import ml_dtypes
    xT = np.ascontiguousarray(input[b].T).astype(ml_dtypes.bfloat16)  # [H, T]

    qcols = slice(g * GD, (g + 1) * GD)
    kcols = slice(H + g * GD, H + (g + 1) * GD)
    vcols = slice(2 * H + g * GD, 2 * H + (g + 1) * GD)
    wq = w_qkv[:, qcols] * scale
    wk = w_qkv[:, kcols]
    wqk = np.ascontiguousarray(np.concatenate([wq, wk], axis=1)).astype(ml_dtypes.bfloat16)
    bqk = np.concatenate([b_qkv[qcols] * scale, b_qkv[kcols]]).astype(np.float32)
    bqkc = np.ascontiguousarray(bqk.reshape(8, 128).T)               # [128, 8]
    wv = np.ascontiguousarray(w_qkv[:, vcols]).astype(ml_dtypes.bfloat16)
    bv = b_qkv[vcols][None, :].astype(np.float32)

    wout = np.ascontiguousarray(w_out[g * GD:(g + 1) * GD, :]).astype(np.float32)
    # b_out on core with g==0 only; zeros on g==1 (partials are summed on host)
    bout = (b_out if g == 0 else np.zeros_like(b_out))[None, :].astype(np.float32)

    padb01 = mask[b].astype(np.float32)                                # [T]
    padb01 = np.ascontiguousarray(padb01.reshape(KC, 128).T)           # [128, KC]

    # 4 causal diagonal mask patterns: delta = 128*dd; valid iff col >= row + delta
    cm = np.empty((128, 4 * 512), dtype=np.float32)
    rr = np.arange(128)[:, None]
    cc = np.arange(512)[None, :]
    for dd in range(4):
        cm[:, dd * 512:(dd + 1) * 512] = np.where(cc >= rr + 128 * dd, 1.0, 0.0)
    cmask = cm.astype(ml_dtypes.bfloat16)

    return {
        "xT": xT, "wqk": wqk, "wv": wv, "bqkc": bqkc, "bv": bv,
        "wout": wout, "bout": bout, "padb01": padb01, "cmask": cmask,
    }


def kernel(input, mask, w_qkv, b_qkv, w_out, b_out):
    from concourse.bass_utils import run_bass_kernel_spmd

    input = np.asarray(input)
    mask = np.asarray(mask)
    w_qkv = np.asarray(w_qkv)
    b_qkv = np.asarray(b_qkv)
    w_out = np.asarray(w_out)
    b_out = np.asarray(b_out)
    nc = _get_nc()
    in_maps = [
        make_core_inputs(input, mask, w_qkv, b_qkv, w_out, b_out, c)
        for c in range(NCORES)
    ]
    res = run_bass_kernel_spmd(nc, in_maps, list(range(NCORES)))
    parts = [res.results[c]["y"] for c in range(NCORES)]
    out = np.stack([parts[2 * b] + parts[2 * b + 1] for b in range(B)])
    return out.astype(np.float32)


if __name__ == "__main__":
    nc = _build_nc()
    print("build ok")



# revision 2
# speedup vs baseline: 1.0718x; 1.0718x over previous
"""Causal multi-head attention (B=4, T=2048, H=1024, 16 heads) on 8 trn2 cores.

Sharding: batch(4) x head-group(2).  Core c -> batch b=c//2, heads g=c%2
(8 heads each): zero-communication data/tensor parallelism.  Each core
computes its QKV projection slice, causal+padding-masked attention for its 8
heads, and a row-parallel slice of the output projection; the two partial
outputs per batch row are summed on the host (row-parallel unshard).

Device algorithm (per core; attention kept transposed so softmax reduces
along the PE contraction dim, all matmuls at 1 cycle/row):
  xT [H, T] bf16 loaded as 32 [128, 512] column-block tiles, t-major, so the
      QK projection (emitted t-tile-outer) starts after ~1.25 MB of DMA
      instead of the full 8 MB input+weight preload.
  QT/KT [512, T] bf16 = wqk^T-slices @ xT  (Q pre-scaled by 1/sqrt(hd) on
      host; bias added on DVE during the PSUM->SBUF move; the wqk slices are
      host-packed ct-major as [128, 1024] tiles so each arrives in one
      long-line DMA just before the stream needs it)
  V [T, 8x65] bf16 = xT^T @ wv, bias via DVE add of a gpsimd-broadcast
      replicated row; a ones column per head; all 65 columns multiplied by
      the key-padding 0/1 mask (per-partition scalar) -> padded keys drop
      out of both the attention numerator and the softmax denominator, so
      the exp needs no mask bias at all.
  per (head, q-tile 512, k-chunk pair 2x128), exact-triangle streaming:
      S^T[k, q] = KT_h[:, chunk].T @ QT_h[:, qtile]     (bf16, f32 PSUM);
      diagonal chunks stream only q >= 128*c (partial-range matmul)
      P^T = exp(S^T)  (ScalarE; full pairs as one [128,1024] op, diagonal
      chunks as exact-range ops so no wasted exp columns)
      causal masking only on the 128x128 diagonal block of diagonal chunks:
      P^T *= upper-tri 0/1 mask on DVE (post-exp, off the ScalarE path)
      o^T[65, q] += V_aug[chunk, head].T @ P^T          (row 64 = denom;
      diagonal chunks accumulate only their valid q sub-range)
  whole-head S streams with the exps trailing on ScalarE, then the dense PV
  stream; V-projection chunks 4..15 and the previous q-tile's output
  projection are braided into the attention stream as PE filler so the HAM
  clock gate stays at 2.4GHz.
  o_scaled = o^T[0:64] * (1/denom) (DVE approx recip, gpsimd broadcast),
  written bf16/DMA-shifted into dense head-pair tiles [128, 512]
  y[t, j] = sum_hp o_dense_hp[:, t].T @ wout_hp[:, j] (bf16) + b_out on DVE,
  stored bf16 (host upcasts and sums the two per-batch partials in f32)
"""

import os
import sys

import numpy as np

sys.path.insert(0, "/opt/trn_rl_repo")

B, T, H = 4, 2048, 1024
NH, HD = 16, 64
NCORES = 8
HPC = 8          # heads per core
GD = HPC * HD    # head dims per core = 512
KC = T // 128    # 16 k-chunks
QT_TILES = T // 512  # 4 q-tiles
HC = H // 128    # 8 h-chunks (contraction for projections)

NEG = -1.0e9


def _build_nc():
    import concourse.bass as bass
    import concourse.tile as tile
    import concourse.mybir as mybir
    from concourse import bacc
    from contextlib import ExitStack

    f32 = mybir.dt.float32
    bf16 = mybir.dt.bfloat16
    EXP = mybir.ActivationFunctionType.Exp

    nc = bacc.Bacc("TRN2", target_bir_lowering=False, debug=False)

    xT_d = nc.dram_tensor("xT", [H, T], bf16, kind="ExternalInput").ap()
    wqkp_d = nc.dram_tensor("wqkp", [128, 8 * 1024], bf16, kind="ExternalInput").ap()
    wv_d = nc.dram_tensor("wv", [H, GD], bf16, kind="ExternalInput").ap()
    bqkc_d = nc.dram_tensor("bqkc", [128, 8], f32, kind="ExternalInput").ap()
    bv_d = nc.dram_tensor("bv", [1, GD], f32, kind="ExternalInput").ap()
    wout_d = nc.dram_tensor("wout", [GD, H], bf16, kind="ExternalInput").ap()
    bout_d = nc.dram_tensor("bout", [1, H], f32, kind="ExternalInput").ap()
    padb01_d = nc.dram_tensor("padb01", [128, KC], f32, kind="ExternalInput").ap()
    tri_d = nc.dram_tensor("tri", [128, 128], bf16, kind="ExternalInput").ap()
    y_d = nc.dram_tensor("y", [T, H], bf16, kind="ExternalOutput").ap()

    def emit_v_proj(nc, bvrep, xtb, wvts, psv, v_sb, padb01_sb, ts, HC, HPC):
        tt, off = ts // 4, (ts % 4) * 128
        for hc in range(HC):
            nc.tensor.matmul(
                psv, xtb[hc][tt][:, off:off + 128], wvts[hc],
                start=(hc == 0), stop=(hc == HC - 1))
        pad_c = padb01_sb[:, ts:ts + 1]
        dst = v_sb[ts].rearrange("p (h c) -> p h c", h=HPC)[:, :, 0:64]
        bsrc = bvrep.rearrange("p (h c) -> p h c", h=HPC)
        srcv = psv.rearrange("p (h c) -> p h c", h=HPC)
        nc.vector.tensor_add(dst, bsrc, srcv)
        nc.vector.tensor_scalar_mul(dst, dst, pad_c)
        onescols = v_sb[ts].rearrange("p (h c) -> p h c", h=HPC)[:, :, 64:65]
        nc.vector.memset(onescols, 1.0)
        nc.vector.tensor_scalar_mul(onescols, onescols, pad_c)

    with ExitStack() as ctx:
        tc = ctx.enter_context(tile.TileContext(nc))

        const = ctx.enter_context(tc.tile_pool(name="const", bufs=1))
        padb01_sb = const.tile([128, KC], f32, name="padb01_sb")
        nc.sync.dma_start(padb01_sb, padb01_d)

        # Persistent activations
        acts = ctx.enter_context(tc.tile_pool(name="acts", bufs=1))
        qk_sb = [acts.tile([128, T], bf16, name=f"qk{i}") for i in range(8)]
        v_sb = [acts.tile([128, HPC * 65], bf16, name=f"v{c}") for c in range(KC)]

        # xt / wv / small consts stay resident through phase 2 (the V
        # projection of chunks 4..15 is braided into the attention stream).
        p1c = ctx.enter_context(tc.tile_pool(name="p1c", bufs=1))
        bqkc_sb = p1c.tile([128, 8], f32, name="bqkc_sb")
        nc.sync.dma_start(bqkc_sb, bqkc_d)
        bv_sb = p1c.tile([1, GD], f32, name="bv_sb")
        nc.sync.dma_start(bv_sb, bv_d)
        bvrep = p1c.tile([128, GD], f32, name="bvrep")
        nc.gpsimd.partition_broadcast(bvrep, bv_sb)

        # x column-block tiles: xtb[hc][tt] is [128, 512].  DMA issue order
        # is what makes the startup fast: first the tt=0 column of x, then
        # the ct-major packed QK weights, then the rest of x, then wv.
        xt_pool = ctx.enter_context(tc.tile_pool(name="xt", bufs=1))
        xtb = [[xt_pool.tile([128, 512], bf16, name=f"xt{i}_{t}")
                for t in range(4)] for i in range(HC)]
        wqk_pool = ctx.enter_context(tc.tile_pool(name="wqkp", bufs=1))
        wqkp_sb = [wqk_pool.tile([128, 1024], bf16, name=f"wqk{ct}")
                   for ct in range(8)]
        wv_pool = ctx.enter_context(tc.tile_pool(name="wvp", bufs=1))
        wvts = [wv_pool.tile([128, GD], bf16, name=f"wv{hc}") for hc in range(HC)]

        for hc in range(HC):
            nc.sync.dma_start(xtb[hc][0], xT_d[hc * 128:(hc + 1) * 128, 0:512])
        for ct in range(8):
            nc.sync.dma_start(wqkp_sb[ct], wqkp_d[:, ct * 1024:(ct + 1) * 1024])
        for tt in range(1, 4):
            for hc in range(HC):
                nc.sync.dma_start(
                    xtb[hc][tt],
                    xT_d[hc * 128:(hc + 1) * 128, tt * 512:(tt + 1) * 512])
        for hc in range(HC):
            nc.sync.dma_start(wvts[hc], wv_d[hc * 128:(hc + 1) * 128, :])

        # ---------------- Phase 1: QK projections + V chunks 0..3 ----------
        with ExitStack() as p1:
            ps1 = p1.enter_context(tc.tile_pool(name="ps1", bufs=4, space="PSUM"))

            # Q^T and K^T: out[col, t] tiles, t-tile-outer so the first psum
            # needs only xtb[*][0] + wqkp[ct] (1.25 MB of DMA).
            for tt in range(4):
                for ct in range(8):
                    ps = ps1.tile([128, 512], f32, tag="ps", name=f"psqk{ct}_{tt}")
                    for hc in range(HC):
                        nc.tensor.matmul(
                            ps, wqkp_sb[ct][:, hc * 128:(hc + 1) * 128],
                            xtb[hc][tt],
                            start=(hc == 0), stop=(hc == HC - 1))
                    nc.vector.tensor_scalar_add(
                        qk_sb[ct][:, tt * 512:(tt + 1) * 512], ps,
                        bqkc_sb[:, ct:ct + 1])

            # V chunks 0..3 (needed by qt0's PV); the rest are braided into
            # the attention stream as PE filler.
            for ts in range(4):
                psv = ps1.tile([128, 512], f32, tag="psv", name=f"psv{ts}")
                emit_v_proj(nc, bvrep, xtb, wvts, psv, v_sb, padb01_sb,
                            ts, HC, HPC)

        # ---------------- Phase 2: attention + output projection ----------------
        with ExitStack() as p2:
            p2c = p2.enter_context(tc.tile_pool(name="p2c", bufs=1))
            tri_sb = p2c.tile([128, 128], bf16, name="tri_sb")
            nc.sync.dma_start(tri_sb, tri_d)
            bout_sb = p2c.tile([1, H], f32, name="bout_sb")
            nc.sync.dma_start(bout_sb, bout_d)
            brep = p2c.tile([128, H], f32, name="brep")
            nc.gpsimd.partition_broadcast(brep, bout_sb)
            wout_sb = [p2c.tile([128, H], bf16, name=f"wo{hp}") for hp in range(4)]
            for hp in range(4):
                nc.sync.dma_start(wout_sb[hp], wout_d[hp * 128:(hp + 1) * 128, :])

            ppool = p2.enter_context(tc.tile_pool(name="pchunks", bufs=14))
            osc_pool = p2.enter_context(tc.tile_pool(name="osc", bufs=2))
            oden_pool = p2.enter_context(tc.tile_pool(name="oden", bufs=8))
            dpool = p2.enter_context(tc.tile_pool(name="dtiles", bufs=3))
            ypool = p2.enter_context(tc.tile_pool(name="ysb", bufs=3))
            ps_s = p2.enter_context(tc.tile_pool(name="ps_s", bufs=2, space="PSUM"))
            ps_o = p2.enter_context(tc.tile_pool(name="ps_o", bufs=2, space="PSUM"))
            ps_y = p2.enter_context(tc.tile_pool(name="ps_y", bufs=2, space="PSUM"))

            def attn_tail(qt, h, opsum, o_dense):
                """softmax denom -> recip -> broadcast -> scale -> dense repack"""
                stage = dpool.tile([65, 512], f32, tag="dstage", name=f"st{qt}_{h}")
                nc.vector.tensor_copy(stage[64:65, :], opsum[64:65, :])
                dp0 = dpool.tile([1, 512], f32, tag="dp0", name=f"dp0_{qt}_{h}")
                nc.sync.dma_start(dp0, stage[64:65, :])
                rp0 = dpool.tile([1, 512], f32, tag="rp0", name=f"rp0_{qt}_{h}")
                nc.vector.reciprocal_approx_fast(rp0, dp0)
                rrep = dpool.tile([64, 512], f32, tag="rrep", name=f"rr{qt}_{h}")
                nc.gpsimd.partition_broadcast(rrep, rp0)
                if h % 2 == 0:
                    nc.vector.tensor_mul(o_dense[0:64, :], rrep, opsum[0:64, :])
                else:
                    o_sc = osc_pool.tile([64, 512], bf16, tag="osc", name=f"osc{qt}_{h}")
                    nc.vector.tensor_mul(o_sc, rrep, opsum[0:64, :])
                    nc.sync.dma_start(o_dense[64:128, :], o_sc)

            def emit_y_tile(qt, j, ts, oden):
                """one output-projection tile for q-tile qt (b_out via preload)"""
                q0 = qt * 512
                ypsum = ps_y.tile([128, 512], f32, tag="y", name=f"y{qt}_{j}_{ts}")
                for hp in range(4):
                    nc.tensor.matmul(
                        ypsum,
                        oden[hp][:, ts * 128:(ts + 1) * 128],
                        wout_sb[hp][:, j * 512:(j + 1) * 512],
                        start=(hp == 0), stop=(hp == 3))
                ysb = ypool.tile([128, 512], bf16, tag="ysb", name=f"ys{qt}_{j}_{ts}")
                nc.vector.tensor_add(ysb, brep[:, j * 512:(j + 1) * 512], ypsum)
                nc.sync.dma_start(
                    y_d[q0 + ts * 128:q0 + (ts + 1) * 128, j * 512:(j + 1) * 512],
                    ysb)

            pending_y = []   # deferred output-projection tiles of the prev q-tile
            deferred_v = list(range(4, KC))   # V chunks braided as PE filler

            for qt in range(QT_TILES):
                q0 = qt * 512
                nk = 4 * (qt + 1)
                oden = []
                for h in range(HPC):
                    if h % 2 == 0:
                        o_dense = oden_pool.tile([128, 512], bf16, tag="od",
                                                 name=f"od{qt}_{h // 2}")
                        oden.append(o_dense)
                    hq = qk_sb[h // 2][(h % 2) * 64:(h % 2) * 64 + 64, q0:q0 + 512]
                    # S^T in two-chunk psum tiles, whole-head S stream first
                    # (exp trails on ScalarE with small frequent PE waits that
                    # don't trip the HAM throttle), then the dense PV stream.
                    # Diagonal chunks stream/exp/accumulate only q >= 128*c.
                    pts = []
                    starts = []
                    for cc in range(nk // 2):
                        spsum = ps_s.tile([128, 1024], f32, tag="s",
                                          name=f"s{qt}_{h}_{cc}")
                        pair_s = []
                        for ci in range(2):
                            c = 2 * cc + ci
                            s = max(0, 128 * (c - 4 * qt))
                            pair_s.append(s)
                            out = spsum[:, ci * 512 + s:(ci + 1) * 512]
                            hk = qk_sb[4 + h // 2][(h % 2) * 64:(h % 2) * 64 + 64,
                                                   c * 128:(c + 1) * 128]
                            nc.tensor.matmul(out, hk, hq[:, s:512],
                                             start=True, stop=True)
                        starts += pair_s
                        pt = ppool.tile([128, 1024], bf16, tag="p",
                                        name=f"p{qt}_{h}_{cc}")
                        if pair_s[1] == 0:
                            nc.scalar.activation(pt, spsum, EXP,
                                                 bias=0.0, scale=1.0)
                        else:
                            for ci in range(2):
                                s = pair_s[ci]
                                nc.scalar.activation(
                                    pt[:, ci * 512 + s:(ci + 1) * 512],
                                    spsum[:, ci * 512 + s:(ci + 1) * 512],
                                    EXP, bias=0.0, scale=1.0)
                        for ci in range(2):
                            c = 2 * cc + ci
                            if c >= 4 * qt:
                                s = pair_s[ci]
                                sl = pt[:, ci * 512 + s:ci * 512 + s + 128]
                                nc.vector.tensor_mul(sl, tri_sb, sl)
                        pts.append(pt)
                        if deferred_v:
                            ts_v = deferred_v.pop(0)
                            psv = ps_y.tile([128, 512], f32, tag="y",
                                            name=f"psvd{ts_v}")
                            emit_v_proj(nc, bvrep, xtb, wvts, psv,
                                        v_sb, padb01_sb, ts_v, HC, HPC)
                        elif cc == 1 and pending_y:
                            pending_y.pop(0)()
                    opsum = ps_o.tile([65, 512], f32, tag="o", name=f"o{qt}_{h}")
                    for c in range(nk):
                        s = starts[c]
                        nc.tensor.matmul(
                            opsum[:, s:512],
                            v_sb[c][:, h * 65:(h + 1) * 65].bitcast(bf16),
                            pts[c // 2][:, (c % 2) * 512 + s:(c % 2) * 512 + 512],
                            start=(c == 0), stop=(c == nk - 1),
                            skip_group_check=True)
                    attn_tail(qt, h, opsum, o_dense)

                for j in range(2):
                    for ts in range(4):
                        pending_y.append(
                            lambda qt=qt, j=j, ts=ts, oden=oden: emit_y_tile(qt, j, ts, oden))

            for fn in pending_y:
                fn()

    nc.compile()
    return nc


_NC_CACHE = None


def _get_nc():
    global _NC_CACHE
    if _NC_CACHE is None:
        _NC_CACHE = _build_nc()
    return _NC_CACHE


def make_core_inputs(input, mask, w_qkv, b_qkv, w_out, b_out, core):
    """Host-side sharding/layout prep for one core."""
    b, g = core // 2, core % 2
    scale = 1.0 / np.sqrt(HD)

    import ml_dtypes
    xT = np.ascontiguousarray(input[b].T).astype(ml_dtypes.bfloat16)  # [H, T]

    qcols = slice(g * GD, (g + 1) * GD)
    kcols = slice(H + g * GD, H + (g + 1) * GD)
    vcols = slice(2 * H + g * GD, 2 * H + (g + 1) * GD)
    wq = w_qkv[:, qcols] * scale
    wk = w_qkv[:, kcols]
    wqk = np.concatenate([wq, wk], axis=1)                            # [H, 2GD]
    # ct-major pack: wqkp[p, ct*1024 + hc*128 + m] = wqk[hc*128 + p, ct*128 + m]
    wqkp = np.ascontiguousarray(
        wqk.reshape(8, 128, 8, 128).transpose(1, 2, 0, 3).reshape(128, 8192)
    ).astype(ml_dtypes.bfloat16)
    bqk = np.concatenate([b_qkv[qcols] * scale, b_qkv[kcols]]).astype(np.float32)
    bqkc = np.ascontiguousarray(bqk.reshape(8, 128).T)               # [128, 8]
    wv = np.ascontiguousarray(w_qkv[:, vcols]).astype(ml_dtypes.bfloat16)
    bv = b_qkv[vcols][None, :].astype(np.float32)

    wout = np.ascontiguousarray(w_out[g * GD:(g + 1) * GD, :]).astype(ml_dtypes.bfloat16)
    # b_out on core with g==0 only; zeros on g==1 (partials are summed on host)
    bout = (b_out if g == 0 else np.zeros_like(b_out))[None, :].astype(np.float32)

    padb01 = mask[b].astype(np.float32)                                # [T]
    padb01 = np.ascontiguousarray(padb01.reshape(KC, 128).T)           # [128, KC]

    # single 128x128 upper-tri (col >= row) causal mask for diagonal blocks
    rr = np.arange(128)[:, None]
    cc = np.arange(128)[None, :]
    tri = np.where(cc >= rr, 1.0, 0.0).astype(ml_dtypes.bfloat16)

    return {
        "xT": xT, "wqkp": wqkp, "wv": wv, "bqkc": bqkc, "bv": bv,
        "wout": wout, "bout": bout, "padb01": padb01, "tri": tri,
    }


def kernel(input, mask, w_qkv, b_qkv, w_out, b_out):
    from concourse.bass_utils import run_bass_kernel_spmd

    input = np.asarray(input)
    mask = np.asarray(mask)
    w_qkv = np.asarray(w_qkv)
    b_qkv = np.asarray(b_qkv)
    w_out = np.asarray(w_out)
    b_out = np.asarray(b_out)
    nc = _get_nc()
    in_maps = [
        make_core_inputs(input, mask, w_qkv, b_qkv, w_out, b_out, c)
        for c in range(NCORES)
    ]
    res = run_bass_kernel_spmd(nc, in_maps, list(range(NCORES)))
    parts = [np.asarray(res.results[c]["y"]).astype(np.float32)
             for c in range(NCORES)]
    out = np.stack([parts[2 * b] + parts[2 * b + 1] for b in range(B)])
    return out.astype(np.float32)


if __name__ == "__main__":
    nc = _build_nc()
    print("build ok")


# revision 9
# speedup vs baseline: 1.1344x; 1.0584x over previous
"""Causal multi-head attention (B=4, T=2048, H=1024, 16 heads) on 8 trn2 cores.

Sharding: batch(4) x head-group(2).  Core c -> batch b=c//2, heads g=c%2
(8 heads each): zero-communication data/tensor parallelism.  Each core
computes its QKV projection slice, causal+padding-masked attention for its 8
heads, and a row-parallel slice of the output projection; the two partial
outputs per batch row are summed on the host (row-parallel unshard).

Device algorithm (per core; attention kept transposed so softmax reduces
along the PE contraction dim, all matmuls at 1 cycle/row):
  DMA issue costs ~650ns each on the sync queue, so inputs arrive as a few
  large transfers (x as 4 column-block tiles via a partition-folding access
  pattern, QK weights host-packed ct-major with the first ct split out) in
  priority order: the first QK psum needs only ~1.25 MB.
  QT/KT [512, T] bf16 = wqk^T-slices @ xT  (Q pre-scaled by 1/sqrt(hd) on
      host; bias added on DVE during the PSUM->SBUF move), t-tile-outer
  V [T, 8x65] bf16 = xT^T @ wv, bias via DVE add of a gpsimd-broadcast
      replicated row; a ones column per head; all 65 columns multiplied by
      the key-padding 0/1 mask (per-partition scalar) -> padded keys drop
      out of both the attention numerator and the softmax denominator.
  per (head, q-tile 512, k-chunk pair 2x128), exact-triangle streaming:
      S^T[k, q] = KT_h[:, chunk].T @ QT_h[:, qtile]     (bf16, f32 PSUM);
      diagonal chunks stream only q >= 128*c (partial-range matmul)
      P^T = exp(S^T)  (ScalarE, ONE op per pair over [s_even:1024) -- the
      never-read gap columns of diagonal pairs hold stale-but-finite psum)
      causal masking only on the 128x128 diagonal block of diagonal chunks:
      P^T *= upper-tri 0/1 mask on DVE (post-exp, off the ScalarE path)
      o^T[65, q] += V_aug[chunk, head].T @ P^T          (row 64 = denom;
      diagonal chunks accumulate only their valid q sub-range)
  whole-head S streams with the exps trailing on ScalarE, then the dense PV
  stream.  PE filler (V-projection chunks and the previous q-tile's output
  projection) is braided just-in-time and spread evenly so the later,
  exp-heavy q-tiles keep the PE busy while ScalarE catches up.
  softmax tail: recip straight off the psum denom row (p64), gpsimd
  partition-broadcast from p64 (no copy/DMA partition shift), scale on DVE.
  Odd head of each pair runs FIRST so the pair's last writer is the even
  head's direct [0:64) write -- the final y tiles never wait on a DMA shift.
  y[t, j] = sum_hp o_dense_hp[:, t].T @ wout_hp[:, j] (bf16) + b_out on DVE,
  stored bf16 (host upcasts and sums the two per-batch partials in f32)
"""

import os
import sys

import numpy as np

sys.path.insert(0, "/opt/trn_rl_repo")

B, T, H = 4, 2048, 1024
NH, HD = 16, 64
NCORES = 8
HPC = 8          # heads per core
GD = HPC * HD    # head dims per core = 512
KC = T // 128    # 16 k-chunks
QT_TILES = T // 512  # 4 q-tiles
HC = H // 128    # 8 h-chunks (contraction for projections)


def _build_nc():
    import concourse.bass as bass
    import concourse.tile as tile
    import concourse.mybir as mybir
    from concourse import bacc
    from contextlib import ExitStack

    f32 = mybir.dt.float32
    bf16 = mybir.dt.bfloat16
    EXP = mybir.ActivationFunctionType.Exp

    nc = bacc.Bacc("TRN2", target_bir_lowering=False, debug=False)

    xT_d = nc.dram_tensor("xT", [H, T], bf16, kind="ExternalInput").ap()
    wqkp_d = nc.dram_tensor("wqkp", [128, 8 * 1024], bf16, kind="ExternalInput").ap()
    wv_d = nc.dram_tensor("wv", [H, GD], bf16, kind="ExternalInput").ap()
    pbq_d = nc.dram_tensor("pbq", [128, KC + 8], f32, kind="ExternalInput").ap()
    bv_d = nc.dram_tensor("bv", [1, GD], f32, kind="ExternalInput").ap()
    wout_d = nc.dram_tensor("wout", [GD, H], bf16, kind="ExternalInput").ap()
    bout_d = nc.dram_tensor("bout", [1, H], f32, kind="ExternalInput").ap()
    tri_d = nc.dram_tensor("tri", [128, 128], bf16, kind="ExternalInput").ap()
    y_d = nc.dram_tensor("y", [T, H], bf16, kind="ExternalOutput").ap()

    def emit_v_proj(nc, bvrep, xtb, wv_sb, psv, v_sb, padb01_sb, ts, HC, HPC):
        tt, off = ts // 4, (ts % 4) * 128
        for hc in range(HC):
            nc.tensor.matmul(
                psv, xtb[tt][:, hc * 512 + off:hc * 512 + off + 128],
                wv_sb[:, hc * 512:(hc + 1) * 512],
                start=(hc == 0), stop=(hc == HC - 1))
        pad_c = padb01_sb[:, ts:ts + 1]
        dst = v_sb[ts].rearrange("p (h c) -> p h c", h=HPC)[:, :, 0:64]
        bsrc = bvrep.rearrange("p (h c) -> p h c", h=HPC)
        srcv = psv.rearrange("p (h c) -> p h c", h=HPC)
        nc.vector.tensor_add(dst, bsrc, srcv)
        nc.vector.tensor_scalar_mul(dst, dst, pad_c)
        onescols = v_sb[ts].rearrange("p (h c) -> p h c", h=HPC)[:, :, 64:65]
        nc.vector.memset(onescols, 1.0)
        nc.vector.tensor_scalar_mul(onescols, onescols, pad_c)

    with ExitStack() as ctx:
        tc = ctx.enter_context(tile.TileContext(nc))

        # Persistent activations
        acts = ctx.enter_context(tc.tile_pool(name="acts", bufs=1))
        qk_sb = [acts.tile([128, T], bf16, name=f"qk{i}") for i in range(8)]
        v_sb = [acts.tile([128, HPC * 65], bf16, name=f"v{c}") for c in range(KC)]

        # Inputs, few big DMAs in priority order: pbq, wqk(ct0), x(tt0),
        # wqk(ct1-7), x(tt1-3), wv, bv.
        p1c = ctx.enter_context(tc.tile_pool(name="p1c", bufs=1))
        pbq_sb = p1c.tile([128, KC + 8], f32, name="pbq_sb")
        padb01_sb = pbq_sb[:, 0:KC]
        bqkc_sb = pbq_sb[:, KC:KC + 8]
        nc.sync.dma_start(pbq_sb, pbq_d)

        xt_pool = ctx.enter_context(tc.tile_pool(name="xt", bufs=1))
        xtb = [xt_pool.tile([128, 8 * 512], bf16, name=f"xt{t}") for t in range(4)]
        wqk_pool = ctx.enter_context(tc.tile_pool(name="wqkp", bufs=1))
        wqkp_sb = wqk_pool.tile([128, 8 * 1024], bf16, name="wqkp_sb")
        wv_pool = ctx.enter_context(tc.tile_pool(name="wvp", bufs=1))
        wv_sb = wv_pool.tile([128, 8 * 512], bf16, name="wv_sb")

        def x_src(tt):
            return xT_d[:, tt * 512:(tt + 1) * 512].rearrange(
                "(h p) t -> p h t", p=128)

        def x_dst(tt):
            return xtb[tt].rearrange("p (h t) -> p h t", h=8)

        nc.sync.dma_start(wqkp_sb[:, 0:1024], wqkp_d[:, 0:1024])
        nc.sync.dma_start(x_dst(0), x_src(0))
        nc.sync.dma_start(wqkp_sb[:, 1024:8192], wqkp_d[:, 1024:8192])
        for tt in range(1, 4):
            nc.sync.dma_start(x_dst(tt), x_src(tt))
        nc.sync.dma_start(wv_sb.rearrange("p (h c) -> p h c", h=8),
                          wv_d.rearrange("(h p) c -> p h c", p=128))
        bv_sb = p1c.tile([1, GD], f32, name="bv_sb")
        nc.sync.dma_start(bv_sb, bv_d)
        bvrep = p1c.tile([128, GD], f32, name="bvrep")
        nc.gpsimd.partition_broadcast(bvrep, bv_sb)

        # ---------------- Phase 1: QK projections + V chunks 0..3 ----------
        with ExitStack() as p1:
            ps1 = p1.enter_context(tc.tile_pool(name="ps1", bufs=4, space="PSUM"))

            # Q^T and K^T: out[col, t] tiles, t-tile-outer so the first psum
            # needs only x(tt0) + wqk(ct0) (1.25 MB of DMA).
            for tt in range(4):
                for ct in range(8):
                    ps = ps1.tile([128, 512], f32, tag="ps", name=f"psqk{ct}_{tt}")
                    for hc in range(HC):
                        nc.tensor.matmul(
                            ps,
                            wqkp_sb[:, ct * 1024 + hc * 128:ct * 1024 + (hc + 1) * 128],
                            xtb[tt][:, hc * 512:(hc + 1) * 512],
                            start=(hc == 0), stop=(hc == HC - 1))
                    nc.vector.tensor_scalar_add(
                        qk_sb[ct][:, tt * 512:(tt + 1) * 512], ps,
                        bqkc_sb[:, ct:ct + 1])

            # V chunks 0..3 (needed by qt0's PV); the rest are braided into
            # the attention stream as PE filler.
            for ts in range(4):
                psv = ps1.tile([128, 512], f32, tag="psv", name=f"psv{ts}")
                emit_v_proj(nc, bvrep, xtb, wv_sb, psv, v_sb, padb01_sb,
                            ts, HC, HPC)

        # ---------------- Phase 2: attention + output projection ----------------
        with ExitStack() as p2:
            p2c = p2.enter_context(tc.tile_pool(name="p2c", bufs=1))
            tri_sb = p2c.tile([128, 128], bf16, name="tri_sb")
            nc.sync.dma_start(tri_sb, tri_d)
            bout_sb = p2c.tile([1, H], f32, name="bout_sb")
            nc.sync.dma_start(bout_sb, bout_d)
            brep = p2c.tile([128, H], f32, name="brep")
            nc.gpsimd.partition_broadcast(brep, bout_sb)
            wout_sb = p2c.tile([128, 4 * H], bf16, name="wo")
            nc.sync.dma_start(wout_sb.rearrange("p (h c) -> p h c", h=4),
                              wout_d.rearrange("(h p) c -> p h c", p=128))

            ppool = p2.enter_context(tc.tile_pool(name="pchunks", bufs=14))
            osc_pool = p2.enter_context(tc.tile_pool(name="osc", bufs=3))
            oden_pool = p2.enter_context(tc.tile_pool(name="oden", bufs=8))
            dpool = p2.enter_context(tc.tile_pool(name="dtiles", bufs=4))
            ypool = p2.enter_context(tc.tile_pool(name="ysb", bufs=3))
            ps_s = p2.enter_context(tc.tile_pool(name="ps_s", bufs=2, space="PSUM"))
            ps_o = p2.enter_context(tc.tile_pool(name="ps_o", bufs=2, space="PSUM"))
            ps_y = p2.enter_context(tc.tile_pool(name="ps_y", bufs=2, space="PSUM"))

            def attn_tail(qt, h, opsum, o_dense):
                """softmax denom -> recip -> broadcast -> scale -> dense repack"""
                stage = dpool.tile([65, 512], f32, tag="dstage", name=f"st{qt}_{h}")
                nc.vector.tensor_copy(stage[64:65, :], opsum[64:65, :])
                dp0 = dpool.tile([1, 512], f32, tag="dp0", name=f"dp0_{qt}_{h}")
                nc.sync.dma_start(dp0, stage[64:65, :])
                rp0 = dpool.tile([1, 512], f32, tag="rp0", name=f"rp0_{qt}_{h}")
                nc.vector.reciprocal_approx_fast(rp0, dp0)
                rrep = dpool.tile([64, 512], f32, tag="rrep", name=f"rr{qt}_{h}")
                nc.gpsimd.partition_broadcast(rrep, rp0)
                if h % 2 == 0:
                    nc.vector.tensor_mul(o_dense[0:64, :], rrep, opsum[0:64, :])
                else:
                    o_sc = osc_pool.tile([64, 512], bf16, tag="osc", name=f"osc{qt}_{h}")
                    nc.vector.tensor_mul(o_sc, rrep, opsum[0:64, :])
                    nc.sync.dma_start(o_dense[64:128, :], o_sc)

            def emit_y_tile(qt, j, ts, oden):
                """one output-projection tile for q-tile qt (b_out via preload)"""
                q0 = qt * 512
                ypsum = ps_y.tile([128, 512], f32, tag="y", name=f"y{qt}_{j}_{ts}")
                for hp in range(4):
                    nc.tensor.matmul(
                        ypsum,
                        oden[hp][:, ts * 128:(ts + 1) * 128],
                        wout_sb[:, hp * 1024 + j * 512:hp * 1024 + (j + 1) * 512],
                        start=(hp == 0), stop=(hp == 3))
                ysb = ypool.tile([128, 512], bf16, tag="ysb", name=f"ys{qt}_{j}_{ts}")
                nc.vector.tensor_add(ysb, brep[:, j * 512:(j + 1) * 512], ypsum)
                nc.sync.dma_start(
                    y_d[q0 + ts * 128:q0 + (ts + 1) * 128, j * 512:(j + 1) * 512],
                    ysb)

            # Just-in-time braided PE filler, spread evenly over each q-tile's
            # pair slots: V chunks arrive one q-tile before their first PV
            # use; y tiles of q-tile qt are braided into qt+1.
            fillers = {qt: [] for qt in range(QT_TILES)}
            for qt in range(QT_TILES - 1):
                for c in range(4 * (qt + 1), 4 * (qt + 2)):
                    fillers[qt].append(
                        lambda c=c: emit_v_proj(
                            nc, bvrep, xtb, wv_sb,
                            ps_y.tile([128, 512], f32, tag="y", name=f"psvd{c}"),
                            v_sb, padb01_sb, c, HC, HPC))
            for qt in range(QT_TILES):
                q0 = qt * 512
                nk = 4 * (qt + 1)
                nslots = (nk // 2) * HPC
                nfill = len(fillers[qt])
                pops = {round((i + 1) * nslots / (nfill + 1)) for i in range(nfill)}
                slot = 0
                oden = []
                for h in [1, 0, 3, 2, 5, 4, 7, 6]:
                    if len(oden) <= h // 2:
                        o_dense = oden_pool.tile([128, 512], bf16, tag="od",
                                                 name=f"od{qt}_{h // 2}")
                        oden.append(o_dense)
                    else:
                        o_dense = oden[h // 2]
                    hq = qk_sb[h // 2][(h % 2) * 64:(h % 2) * 64 + 64, q0:q0 + 512]
                    # S^T in two-chunk psum tiles, whole-head S stream first
                    # (exp trails on ScalarE), then the dense PV stream.
                    # Diagonal chunks stream/accumulate only q >= 128*c.
                    pts = []
                    starts = []
                    for cc in range(nk // 2):
                        spsum = ps_s.tile([128, 1024], f32, tag="s",
                                          name=f"s{qt}_{h}_{cc}")
                        pair_s = []
                        for ci in range(2):
                            c = 2 * cc + ci
                            s = max(0, 128 * (c - 4 * qt))
                            pair_s.append(s)
                            out = spsum[:, ci * 512 + s:(ci + 1) * 512]
                            hk = qk_sb[4 + h // 2][(h % 2) * 64:(h % 2) * 64 + 64,
                                                   c * 128:(c + 1) * 128]
                            nc.tensor.matmul(out, hk, hq[:, s:512],
                                             start=True, stop=True)
                        starts += pair_s
                        pt = ppool.tile([128, 1024], bf16, tag="p",
                                        name=f"p{qt}_{h}_{cc}")
                        s0 = pair_s[0]
                        nc.scalar.activation(pt[:, s0:1024], spsum[:, s0:1024],
                                             EXP, bias=0.0, scale=1.0)
                        for ci in range(2):
                            c = 2 * cc + ci
                            if c >= 4 * qt:
                                s = pair_s[ci]
                                sl = pt[:, ci * 512 + s:ci * 512 + s + 128]
                                nc.vector.tensor_mul(sl, tri_sb, sl)
                        pts.append(pt)
                        slot += 1
                        # evenly spread braided filler over this q-tile
                        if slot in pops and fillers[qt]:
                            fillers[qt].pop(0)()
                    opsum = ps_o.tile([65, 512], f32, tag="o", name=f"o{qt}_{h}")
                    for c in range(nk):
                        s = starts[c]
                        nc.tensor.matmul(
                            opsum[:, s:512],
                            v_sb[c][:, h * 65:(h + 1) * 65].bitcast(bf16),
                            pts[c // 2][:, (c % 2) * 512 + s:(c % 2) * 512 + 512],
                            start=(c == 0), stop=(c == nk - 1),
                            skip_group_check=True)
                    attn_tail(qt, h, opsum, o_dense)

                if qt < QT_TILES - 1:
                    for j in range(2):
                        for ts in range(4):
                            fillers[qt + 1].append(
                                lambda qt=qt, j=j, ts=ts, oden=oden:
                                    emit_y_tile(qt, j, ts, oden))
                else:
                    for j in range(2):
                        for ts in range(4):
                            emit_y_tile(qt, j, ts, oden)

    nc.compile()
    return nc


_NC_CACHE = None


def _get_nc():
    global _NC_CACHE
    if _NC_CACHE is None:
        _NC_CACHE = _build_nc()
    return _NC_CACHE


def make_core_inputs(input, mask, w_qkv, b_qkv, w_out, b_out, core):
    """Host-side sharding/layout prep for one core."""
    b, g = core // 2, core % 2
    scale = 1.0 / np.sqrt(HD)

    import ml_dtypes
    xT = np.ascontiguousarray(input[b].T).astype(ml_dtypes.bfloat16)  # [H, T]

    qcols = slice(g * GD, (g + 1) * GD)
    kcols = slice(H + g * GD, H + (g + 1) * GD)
    vcols = slice(2 * H + g * GD, 2 * H + (g + 1) * GD)
    wq = w_qkv[:, qcols] * scale
    wk = w_qkv[:, kcols]
    wqk = np.concatenate([wq, wk], axis=1)                            # [H, 2GD]
    # ct-major pack: wqkp[p, ct*1024 + hc*128 + m] = wqk[hc*128 + p, ct*128 + m]
    wqkp = np.ascontiguousarray(
        wqk.reshape(8, 128, 8, 128).transpose(1, 2, 0, 3).reshape(128, 8192)
    ).astype(ml_dtypes.bfloat16)
    bqk = np.concatenate([b_qkv[qcols] * scale, b_qkv[kcols]]).astype(np.float32)
    bqkc = np.ascontiguousarray(bqk.reshape(8, 128).T)               # [128, 8]
    wv = np.ascontiguousarray(w_qkv[:, vcols]).astype(ml_dtypes.bfloat16)
    bv = b_qkv[vcols][None, :].astype(np.float32)

    wout = np.ascontiguousarray(w_out[g * GD:(g + 1) * GD, :]).astype(ml_dtypes.bfloat16)
    # b_out on core with g==0 only; zeros on g==1 (partials are summed on host)
    bout = (b_out if g == 0 else np.zeros_like(b_out))[None, :].astype(np.float32)

    padb01 = mask[b].astype(np.float32)                                # [T]
    padb01 = np.ascontiguousarray(padb01.reshape(KC, 128).T)           # [128, KC]
    pbq = np.concatenate([padb01, bqkc], axis=1).astype(np.float32)    # [128, 24]

    # single 128x128 upper-tri (col >= row) causal mask for diagonal blocks
    rr = np.arange(128)[:, None]
    cc = np.arange(128)[None, :]
    tri = np.where(cc >= rr, 1.0, 0.0).astype(ml_dtypes.bfloat16)

    return {
        "xT": xT, "wqkp": wqkp, "wv": wv, "pbq": pbq, "bv": bv,
        "wout": wout, "bout": bout, "tri": tri,
    }


def kernel(input, mask, w_qkv, b_qkv, w_out, b_out):
    from concourse.bass_utils import run_bass_kernel_spmd

    input = np.asarray(input)
    mask = np.asarray(mask)
    w_qkv = np.asarray(w_qkv)
    b_qkv = np.asarray(b_qkv)
    w_out = np.asarray(w_out)
    b_out = np.asarray(b_out)
    nc = _get_nc()
    in_maps = [
        make_core_inputs(input, mask, w_qkv, b_qkv, w_out, b_out, c)
        for c in range(NCORES)
    ]
    res = run_bass_kernel_spmd(nc, in_maps, list(range(NCORES)))
    parts = [np.asarray(res.results[c]["y"]).astype(np.float32)
             for c in range(NCORES)]
    out = np.stack([parts[2 * b] + parts[2 * b + 1] for b in range(B)])
    return out.astype(np.float32)


if __name__ == "__main__":
    nc = _build_nc()
    print("build ok")
